# revision 26
# baseline (speedup 1.0000x reference)
"""Multi-head self-attention on 8 Trainium2 NeuronCores.

Problem: x[2, 4096, 768], Wq/Wk/Wv[768, 512], Wout[512, 768], b_out[768]
  q,k,v = heads(x@W*); S = qk^T/8; P = softmax(S); out = (P v) @ Wout + b_out
Sharding: 16 (batch, head) pairs -> 8 cores, 2 heads each (core i: batch i//4,
  heads 2*(i%4), 2*(i%4)+1). Each core holds its weight slices and computes a
  partial y^T[768, 4096]; host sums the 4 per-batch partials and adds b_out.

Device dataflow (all matmuls bf16, fp32 PSUM accumulation):
  x^T (transposed+cast on host)  ->  q^T,k^T [128, 4096]  (W stationary;
    two adjacent 512-col chunks share one PSUM slot and one 1024-wide copy)
  v natural [4096, 128] via x^T-stationary matmuls (4 j-tiles per PSUM
    slot/copy), stored with a ones column per head (v_ext[j, h, 0:65])
  S^T[j,i] both heads per j-tile via row-tiled (tile_position (0,0)/(64,0))
    K=64 matmul pairs into [128, 1024] PSUM groups (3 slots, 6 banks)
  P^T = exp(S^T/8): ~20/32 groups per i-chunk on ScalarE (table exp, exact
    to bf16), the KDVEGI groups on DVE as a 1-op Schraudolph bit-trick
    (int16(a*s+b) bitcast to bf16, ~1.8% rms) -- splitting the softmax
    across both PSUM-capable engines is what unblocks the PE.
  AV with P^T STATIONARY (lhsT): per (head, i-sub 128) chain over 8 j-tiles
    per quarter, rhs = v_ext[j, h, 0:65] -> o^T[i, 0:64] and Z[i] in col 64
    of the same PSUM accumulation (softmax denominator is free); quarter
    partials combine on DVE (seed copy on ScalarE), releasing P^T slots
    early for the next chunk's exp.
  normalize: rz4 = 1/Z (one DVE reciprocal per head), o^T scaled via one
    broadcast tensor_tensor per head into bf16 oTn; PE-transpose -> o[d, i]
  out-proj: single K=128 matmul per 128-row slice of Wout; tail of chunk i
    drains as fine-grained fillers behind chunk i+1's S/exp stream.
  Engine budget per core (sim): PE ~221 us (the wall), ScalarE ~166,
    DVE ~166; PSUM->SBUF traffic must stay on ScalarE/DVE (Pool/GpSimd has
    no PSUM port). fp8 DoubleRow for S was tried and reverted: e4m3 q/k
    quantization alone costs ~3e-2 max-rel error (over the 2e-2 budget).
"""
import os
import numpy as np
import ml_dtypes

ABLATE = set(os.environ.get("KABLATE", "").split(","))
KITER = int(os.environ.get("KITER", "1"))
# softmax groups (of 8 per i-chunk quarter) whose exp runs on DVE not ScalarE
DVE_GI = frozenset(
    int(x) for x in os.environ.get("KDVEGI", "1,4,7").split(",") if x != "")
KCOPY = os.environ.get("KCOPY", "dve")   # engine for qk/v/osb bulk copies
KYCOPY = os.environ.get("KYCOPY", "dve")  # engine for y PSUM->SBUF copies
KTRDMA = os.environ.get("KTRDMA", "0") == "1"  # tail transposes via DMA XBAR
# split-tile exp: Act handles [0:KSPLIT) of each 1024-wide softmax tile and
# DVE-Schraudolph the rest (0 = whole-tile assignment via KDVEGI instead)
KSPLIT = int(os.environ.get("KSPLIT", "0"))

import concourse.bass as bass
import concourse.mybir as mybir
import concourse.tile as tile
from concourse import bacc
from concourse.bass_utils import run_bass_kernel_spmd

BF16 = ml_dtypes.bfloat16
F32 = mybir.dt.float32
BF = mybir.dt.bfloat16
F8 = mybir.dt.float8e4

B, N, QDIM = 2, 4096, 768
H, D = 8, 64
KT = QDIM // 128          # 6 contraction tiles
NCH = N // 512            # 8 i-chunks
NJT = N // 128            # 32 j-tiles
SCALE = D ** -0.5         # 1/8

# DVE 1-op "Schraudolph" exp for offloaded softmax tiles: bf16's bit layout
# is sign(1)|exp(8)|man(7), so int16 i = round(a*z + b) with a = 128/ln2,
# b = 127*128 + sigma reinterpreted as bf16 gives 2^(z/ln2) with a piecewise-
# linear mantissa (rel err ~1.8% rms, ~4% max; sigma = -7.5 centers the mean,
# +0.5 compensates truncate-on-convert). The fp32 PSUM read and int16 SBUF
# write (bitcast over the bf16 P tile) happen in one tensor_scalar pass.
import math
SCHR_A = 128 / math.log(2) / 8     # x SCALE folded in
SCHR_B = 127 * 128 - 7.5 + 0.5


def _body(ctx, tc):
    nc = tc.nc

    xT = nc.dram_tensor("xT", [QDIM, N], BF, kind="ExternalInput").ap()
    wq = nc.dram_tensor("wq", [QDIM, 128], BF, kind="ExternalInput").ap()
    wk = nc.dram_tensor("wk", [QDIM, 128], BF, kind="ExternalInput").ap()
    wv = nc.dram_tensor("wv", [QDIM, 128], BF, kind="ExternalInput").ap()
    wout = nc.dram_tensor("wout", [128, QDIM], BF, kind="ExternalInput").ap()
    ident = nc.dram_tensor("ident", [128, 128], BF, kind="ExternalInput").ap()
    yT = nc.dram_tensor("yT", [QDIM, N], F32, kind="ExternalOutput").ap()

    xT_r = xT.rearrange("(k p) n -> p k n", p=128)
    wq_r = wq.rearrange("(k p) m -> p k m", p=128)
    wk_r = wk.rearrange("(k p) m -> p k m", p=128)
    wv_r = wv.rearrange("(k p) m -> p k m", p=128)
    wout_r = wout.rearrange("p (k f) -> p k f", f=128)
    yT_r = yT.rearrange("(m p) n -> m p n", p=128)

    # ---- static SBUF ----
    singles = ctx.enter_context(tc.tile_pool(name="singles", bufs=1))
    xT_sb = singles.tile([128, KT, N], BF, name="xT_sb", tag="xT_sb")
    wq_sb = singles.tile([128, KT, 128], BF, name="wq_sb", tag="wq_sb")
    wk_sb = singles.tile([128, KT, 128], BF, name="wk_sb", tag="wk_sb")
    wv_sb = singles.tile([128, KT, 128], BF, name="wv_sb", tag="wv_sb")
    wout_sb = singles.tile([128, KT, 128], BF, name="wout_sb", tag="wout_sb")
    id_sb = singles.tile([128, 128], BF, name="id_sb", tag="id_sb")
    qT_sb = singles.tile([128, N], BF, name="qT_sb", tag="qT_sb")
    kT_sb = singles.tile([128, N], BF, name="kT_sb", tag="kT_sb")
    # v per j-tile and head, with a trailing ones column: [j, jt, h, 0:64]=v,
    # [j, jt, h, 64]=1.0 (the AV rhs [j, 65] then accumulates Z in out col 64).
    # Double-buffered across KITER iterations so iteration n+1's v projection
    # does not serialize behind iteration n's AV reads.
    v_sb = singles.tile([128, 2, NJT, 2, 65], BF, name="v_sb", tag="v_sb")

    for k in range(KT):
        for q4 in range(4):
            qs = bass.ts(q4, N // 4)
            nc.sync.dma_start(out=xT_sb[:, k, qs], in_=xT_r[:, k, qs])
    nc.sync.dma_start(out=wq_sb, in_=wq_r)
    nc.sync.dma_start(out=wk_sb, in_=wk_r)
    nc.sync.dma_start(out=wv_sb, in_=wv_r)
    nc.sync.dma_start(out=wout_sb, in_=wout_r)
    nc.sync.dma_start(out=id_sb, in_=ident)
    nc.vector.memset(v_sb[:, :, :, :, 64], 1.0)

    psA = ctx.enter_context(tc.tile_pool(name="psA", bufs=3, space="PSUM"))
    psB = ctx.enter_context(tc.tile_pool(name="psB", bufs=2, space="PSUM"))
    ptp = ctx.enter_context(tc.tile_pool(name="ptp", bufs=6))
    sm = ctx.enter_context(tc.tile_pool(name="sm", bufs=2))
    yp = ctx.enter_context(tc.tile_pool(name="yp", bufs=3))

    from collections import deque
    fillers = deque()
    pending = []
    for _it in range(KITER):
        _compute(nc, psA, psB, ptp, sm, yp,
                 xT_sb, wq_sb, wk_sb, wv_sb, wout_sb, id_sb, qT_sb, kT_sb,
                 v_sb, yT_r, fillers, pending, _it)
    while fillers:
        fillers.popleft()[1]()
    while pending:
        if "tail" in ABLATE:
            break
        ich, oTs, tfn = pending.pop(0)
        for _, u in tfn(ich, oTs):
            u()


def _compute(nc, psA, psB, ptp, sm, yp, xT_sb, wq_sb, wk_sb, wv_sb,
             wout_sb, id_sb, qT_sb, kT_sb, v_sb, yT_r, fillers, pending, it):
    Exp = mybir.ActivationFunctionType.Exp
    Mul = mybir.AluOpType.mult
    Add = mybir.AluOpType.add
    I16 = mybir.dt.int16
    vb = it % 2

    # ---- 1-op Schraudolph exp on DVE for offloaded softmax tiles ----
    def dve_exp(st, out_ap):
        nc.vector.tensor_scalar(out=out_ap.bitcast(I16), in0=st,
                                scalar1=SCHR_A, scalar2=SCHR_B,
                                op0=Mul, op1=Add)

    # ---- projections: q^T, k^T = W^T @ x^T. Chunk 0's first S group needs
    # kT chunks 0-1 and qT chunk 0 (emitted inline); the other 13 chunks
    # drain as fillers, deadline-interleaved with the previous iteration's
    # leftover AV units (kT chunk 2q..2q+1 gates this chunk's quarter q;
    # AV(7) blocks of 8 gate the P^T quarter-slot reuse). ----
    def proj_qk(w_sb, dst, nch, nch2=None):
        # one "s" slot and one copy for up to two adjacent 512-col chunks
        pq = psA.tile([128, 1024], F32, tag="s", name="pq")
        chunks = (nch,) if nch2 is None else (nch, nch2)
        for ci, n in enumerate(chunks):
            for k in range(KT):
                nc.tensor.matmul(pq[:, ci * 512:ci * 512 + 512],
                                 lhsT=w_sb[:, k, :],
                                 rhs=xT_sb[:, k, bass.ts(n, 512)],
                                 start=(k == 0), stop=(k == KT - 1))
        cp = nc.scalar.copy if KCOPY == "act" else nc.vector.tensor_copy
        w = len(chunks) * 512
        cp(out=dst[:, nch * 512:nch * 512 + w], in_=pq[:, 0:w])

    if it == 0:
        # later iterations' kT0/kT1/qT0 recompute is hoisted into the
        # previous iteration's chunk-7 queue (prelude below)
        proj_qk(wk_sb, kT_sb, 0)
        proj_qk(wk_sb, kT_sb, 1)
        proj_qk(wq_sb, qT_sb, 0)
    punits = ([(2.6, lambda n=n: proj_qk(wk_sb, kT_sb, n, n + 1))
               for n in range(2, NCH, 2)]
              + [(2.6, lambda n=n: proj_qk(wq_sb, qT_sb, n, n + 1))
                 for n in range(1, NCH - 1, 2)]
              + [(1.3, lambda: proj_qk(wq_sb, qT_sb, NCH - 1))])
    left = list(fillers)
    fillers.clear()
    if len(left) >= 32:  # steady seam: [tail ...][AV(7) x32]
        tl, av = left[:len(left) - 32], left[len(left) - 32:]
        fillers.extend([punits[0]] + av[0:8]
                       + [punits[1], punits[2]] + av[8:16]
                       + [punits[3], punits[4]] + av[16:24]
                       + [punits[5]] + av[24:32] + tl + punits[6:])
    else:
        fillers.extend(left + punits)

    # ---- projection: v natural (x^T tiles stationary); emitted inside
    # i-chunk 0 per half, right before the AV that first consumes it ----
    def proj_v(jt4):
        # four j-tiles share one "s" slot and one copy
        pv = psA.tile([128, 1024], F32, tag="s", name="pv")
        for t in range(4):
            js = bass.ts(jt4 * 4 + t, 128)
            for k in range(KT):
                nc.tensor.matmul(pv[:, t * 128:t * 128 + 128],
                                 lhsT=xT_sb[:, k, js],
                                 rhs=wv_sb[:, k, :],
                                 start=(k == 0), stop=(k == KT - 1))
        cp = nc.scalar.copy if KCOPY == "act" else nc.vector.tensor_copy
        cp(out=v_sb[:, vb, jt4 * 4:jt4 * 4 + 4, :, 0:64],
           in_=pv[:, 0:512].rearrange("p (t h d) -> p t h d", t=4, h=2))

    # ---- tail: normalize by Z (PSUM col 64), transpose, out-project.
    # Returns a list of closures (filler units) so the PE work interleaves
    # with the next chunk's S/exp stream instead of blocking it. ----
    def tail_units(ich, oTs):
        ics = bass.ts(ich, 512)
        holder = {}

        def norm():
            oTn = holder["oTn"] = sm.tile([128, 4, 128], BF, tag="oTn",
                                          name="oTn")
            Mul = mybir.AluOpType.mult
            for hh in range(2):
                rz4 = sm.tile([128, 4], F32, tag="rz", name="rz4", bufs=4)
                nc.vector.reciprocal(out=rz4, in_=oTs[0][hh][:, :, 64])
                nc.vector.tensor_tensor(
                    out=oTn[:, :, hh * 64:hh * 64 + 64],
                    in0=oTs[0][hh][:, :, 0:64],
                    in1=rz4.rearrange("p (f o) -> p f o", o=1).broadcast_to([128, 4, 64]),
                    op=Mul)
            holder["osb"] = sm.tile([128, 512], BF, tag="osb", name="osb")

        def tr(isub):
            if KTRDMA:
                nc.sync.dma_start(out=holder["osb"][:, bass.ts(isub, 128)],
                                  in_=holder["oTn"][:, isub, :],
                                  transpose=True)
                return
            tps = psB.tile([128, 128], BF, tag="o", name="tps")
            nc.tensor.transpose(tps, holder["oTn"][:, isub, :], id_sb)
            cp = nc.scalar.copy if KCOPY == "act" else nc.vector.tensor_copy
            cp(out=holder["osb"][:, bass.ts(isub, 128)], in_=tps)

        def proj(m):
            py = psB.tile([128, 512], F32, tag="o", name="py")
            nc.tensor.matmul(py, lhsT=wout_sb[:, m, :], rhs=holder["osb"],
                             start=True, stop=True)
            yb = yp.tile([128, 512], F32, tag="yb", name="yb")
            if KYCOPY == "act":
                nc.scalar.copy(out=yb, in_=py)
            else:
                nc.vector.tensor_copy(out=yb, in_=py)
            nc.sync.dma_start(out=yT_r[m, :, ics], in_=yb)

        return ([(0.05, norm)]
                + [(0.2, lambda i=i: tr(i)) for i in range(4)]
                + [(0.3, lambda m=m: proj(m)) for m in range(KT)])

    # ---- AV: P^T stationary (lhsT), rhs = [v | 1], Z lands in out col 64.
    # One quarter-pass per unit: the 4 i-sub accumulation chains of a head
    # run sequentially (PSUM allows only one open accumulation group per
    # bank), then DVE folds the quarter's partial o^T into an SBUF
    # accumulator. The P^T quarter-tile is fully consumed after its two
    # units, releasing its slot early for the next chunk's exp. ----
    def av_units(qts, oSs):
        def avq(q, hh):
            if "av" in ABLATE:
                return
            if q == 0 and hh == 0:
                oSs.append([sm.tile([128, 4, 65], F32, tag="oS", name="oS",
                                    bufs=4)
                            for _ in range(2)])
                oSs.append([psB.tile([128, 4, 68], F32, tag="o", name="oT")
                            for _ in range(2)])
            oS, oT = oSs[0][hh], oSs[1][hh]
            for isub in range(4):
                io = hh * 512 + isub * 128
                for jl in range(8):
                    nc.tensor.matmul(
                        oT[:, isub, 0:65],
                        lhsT=qts[q][:, jl, io:io + 128],
                        rhs=v_sb[:, vb, q * 8 + jl, hh, :],
                        start=(jl == 0), stop=(jl == 7))
            if q == 0:
                cp = (nc.scalar.copy if KOSCOPY == "act"
                      else nc.vector.tensor_copy)
                cp(out=oS, in_=oT[:, :, 0:65])
            else:
                nc.vector.tensor_add(oS, oS, oT[:, :, 0:65])
        return [(1.0, lambda q=q, hh=hh: avq(q, hh))
                for q in range(4) for hh in range(2)]

    # ---- attention per i-chunk; AV(c) and tail(c-1) drain as fillers
    # between chunk c+1's S/exp groups, budgeted so the PE work emitted per
    # group stays within that group's ScalarE time (values in ~us of PE) ----
    def drain(budget):
        while fillers and budget > 0:
            cost, fn = fillers.popleft()
            fn()
            budget -= cost

    for ich in range(NCH):
        ics = bass.ts(ich, 512)
        drain(KSEAM)
        qts = []
        for q in range(4):
            ptb = ptp.tile([128, 8, 1024], BF, tag="pt", name="ptb")
            ptf = ptb.rearrange("p a b -> p (a b)")
            qts.append(ptb)
            g = 0
            for gi, csz in enumerate((2,) * 8):
                st = psA.tile([128, csz * 512], F32, tag="s", name="st")
                for s in range(csz):
                    jl, h = (g + s) // 2, (g + s) % 2
                    jt = q * 8 + jl
                    js = bass.ts(jt, 128)
                    if "s" in ABLATE:
                        continue
                    nc.tensor.matmul(st[:, bass.ts(s, 512)],
                                     lhsT=kT_sb[h * 64:h * 64 + 64, js],
                                     rhs=qT_sb[h * 64:h * 64 + 64, ics],
                                     start=True, stop=True,
                                     tile_position=(h * 64, 0))
                if "exp" not in ABLATE:
                    dst = ptf[:, g * 512:(g + csz) * 512]
                    if KSPLIT and "dve" not in ABLATE:
                        nc.scalar.activation(out=dst[:, 0:KSPLIT],
                                             in_=st[:, 0:KSPLIT],
                                             func=Exp, scale=SCALE)
                        dve_exp(st[:, KSPLIT:csz * 512],
                                dst[:, KSPLIT:csz * 512])
                    elif gi in DVE_GI and "dve" not in ABLATE:
                        dve_exp(st, dst)
                    else:
                        nc.scalar.activation(out=dst, in_=st,
                                             func=Exp, scale=SCALE)
                g += csz
                drain(0.55)
            if ich == NCH - 1 and q == 0:
                # prelude: next iteration's first projections, emitted after
                # this iteration's last reader of kT chunks 0-1 / qT chunk 0
                fillers.extendleft([
                    (1.3, lambda: proj_qk(wq_sb, qT_sb, 0)),
                    (2.6, lambda: proj_qk(wk_sb, kT_sb, 0, 1))])
        if pending and "tail" not in ABLATE:
            pich, poTs, ptfn = pending.pop(0)
            fillers.extend(ptfn(pich, poTs))
        oSs = []
        avs = av_units(qts, oSs)
        if ich == 0:
            # refresh v (quarter q's 8 tiles) right before the first AV
            # quarter-pass that consumes it
            avs = [u for q in range(4)
                   for u in ([(1.7, lambda jt4=jt4: proj_v(jt4))
                              for jt4 in (q * 2, q * 2 + 1)]
                             + avs[2 * q:2 * q + 2])]
        fillers.extend(avs)
        pending.append((ich, oSs, tail_units))


_CACHE = {}


def _build():
    if "nc" not in _CACHE:
        nc = bacc.Bacc("TRN2", target_bir_lowering=False, debug=False,
                       num_devices=8)
        from contextlib import ExitStack
        with tile.TileContext(nc) as tc:
            with ExitStack() as ctx:
                _body(ctx, tc)
        nc.compile()
        _CACHE["nc"] = nc
    return _CACHE["nc"]


def make_in_maps(x, Wq, Wk, Wv, Wout):
    in_maps = []
    ident = np.eye(128, dtype=BF16)
    for core in range(8):
        b = core // 4
        sl = slice((core % 4) * 128, (core % 4) * 128 + 128)
        in_maps.append({
            "xT": x[b].T.astype(BF16),
            "wq": Wq[:, sl].astype(BF16),
            "wk": Wk[:, sl].astype(BF16),
            "wv": Wv[:, sl].astype(BF16),
            "wout": Wout[sl, :].astype(BF16),
            "ident": ident,
        })
    return in_maps


def kernel(x, Wq, Wk, Wv, Wout, b_out):
    x, Wq, Wk, Wv, Wout, b_out = (np.asarray(a) for a in
                                  (x, Wq, Wk, Wv, Wout, b_out))
    nc = _build()
    in_maps = make_in_maps(x, Wq, Wk, Wv, Wout)
    res = run_bass_kernel_spmd(nc, in_maps, core_ids=list(range(8)))
    y = np.zeros((B, N, QDIM), np.float32)
    for core in range(8):
        y[core // 4] += res.results[core]["yT"].T
    y += b_out.astype(np.float32)
    return y



# revision 27
# speedup vs baseline: 1.0121x; 1.0121x over previous
"""Multi-head self-attention on 8 Trainium2 NeuronCores.

Problem: x[2, 4096, 768], Wq/Wk/Wv[768, 512], Wout[512, 768], b_out[768]
  q,k,v = heads(x@W*); S = qk^T/8; P = softmax(S); out = (P v) @ Wout + b_out
Sharding: 16 (batch, head) pairs -> 8 cores, 2 heads each (core i: batch i//4,
  heads 2*(i%4), 2*(i%4)+1). Each core holds its weight slices and computes a
  partial y^T[768, 4096]; host sums the 4 per-batch partials and adds b_out.

Device dataflow (all matmuls bf16, fp32 PSUM accumulation):
  x^T (transposed+cast on host)  ->  q^T,k^T [128, 4096]  (W stationary;
    two adjacent 512-col chunks share one PSUM slot and one 1024-wide copy)
  v natural [4096, 128] via x^T-stationary matmuls (4 j-tiles per PSUM
    slot/copy), stored with a ones column per head (v_ext[j, h, 0:65])
  S^T[j,i] both heads per j-tile via row-tiled (tile_position (0,0)/(64,0))
    K=64 matmul pairs into [128, 1024] PSUM groups (3 slots, 6 banks)
  P^T = exp(S^T/8): ~20/32 groups per i-chunk on ScalarE (table exp, exact
    to bf16), the KDVEGI groups on DVE as a 1-op Schraudolph bit-trick
    (int16(a*s+b) bitcast to bf16, ~1.8% rms) -- splitting the softmax
    across both PSUM-capable engines is what unblocks the PE.
  AV with P^T STATIONARY (lhsT): per (head, i-sub 128) chain over 8 j-tiles
    per quarter, rhs = v_ext[j, h, 0:65] -> o^T[i, 0:64] and Z[i] in col 64
    of the same PSUM accumulation (softmax denominator is free); quarter
    partials combine on DVE (seed copy on ScalarE), releasing P^T slots
    early for the next chunk's exp.
  normalize: rz4 = 1/Z (one DVE reciprocal per head), o^T scaled via one
    broadcast tensor_tensor per head into bf16 oTn; PE-transpose -> o[d, i]
  out-proj: single K=128 matmul per 128-row slice of Wout; tail of chunk i
    drains as fine-grained fillers behind chunk i+1's S/exp stream.
  Engine budget per core (sim): PE ~221 us (the wall), ScalarE ~166,
    DVE ~166; PSUM->SBUF traffic must stay on ScalarE/DVE (Pool/GpSimd has
    no PSUM port). fp8 DoubleRow for S was tried and reverted: e4m3 q/k
    quantization alone costs ~3e-2 max-rel error (over the 2e-2 budget).
"""
import os
import numpy as np
import ml_dtypes

ABLATE = set(os.environ.get("KABLATE", "").split(","))
KITER = int(os.environ.get("KITER", "1"))
# softmax groups (of 8 per i-chunk quarter) whose exp runs on DVE not ScalarE
DVE_GI = frozenset(
    int(x) for x in os.environ.get("KDVEGI", "2,5,7").split(",") if x != "")
KCOPY = os.environ.get("KCOPY", "dve")   # engine for qk/v/osb bulk copies
KYCOPY = os.environ.get("KYCOPY", "dve")  # engine for y PSUM->SBUF copies
KTRDMA = os.environ.get("KTRDMA", "0") == "1"  # tail transposes via DMA XBAR
# split-tile exp: Act handles [0:KSPLIT) of each 1024-wide softmax tile and
# DVE-Schraudolph the rest (0 = whole-tile assignment via KDVEGI instead)
KSPLIT = int(os.environ.get("KSPLIT", "0"))

import concourse.bass as bass
import concourse.mybir as mybir
import concourse.tile as tile
from concourse import bacc
from concourse.bass_utils import run_bass_kernel_spmd

BF16 = ml_dtypes.bfloat16
F32 = mybir.dt.float32
BF = mybir.dt.bfloat16
F8 = mybir.dt.float8e4

B, N, QDIM = 2, 4096, 768
H, D = 8, 64
KT = QDIM // 128          # 6 contraction tiles
NCH = N // 512            # 8 i-chunks
NJT = N // 128            # 32 j-tiles
SCALE = D ** -0.5         # 1/8

# DVE 1-op "Schraudolph" exp for offloaded softmax tiles: bf16's bit layout
# is sign(1)|exp(8)|man(7), so int16 i = round(a*z + b) with a = 128/ln2,
# b = 127*128 + sigma reinterpreted as bf16 gives 2^(z/ln2) with a piecewise-
# linear mantissa (rel err ~1.8% rms, ~4% max; sigma = -7.5 centers the mean,
# +0.5 compensates truncate-on-convert). The fp32 PSUM read and int16 SBUF
# write (bitcast over the bf16 P tile) happen in one tensor_scalar pass.
import math
SCHR_A = 128 / math.log(2) / 8     # x SCALE folded in
SCHR_B = 127 * 128 - 7.5 + 0.5


def _body(ctx, tc):
    nc = tc.nc

    xT = nc.dram_tensor("xT", [QDIM, N], BF, kind="ExternalInput").ap()
    wq = nc.dram_tensor("wq", [QDIM, 128], BF, kind="ExternalInput").ap()
    wk = nc.dram_tensor("wk", [QDIM, 128], BF, kind="ExternalInput").ap()
    wv = nc.dram_tensor("wv", [QDIM, 128], BF, kind="ExternalInput").ap()
    wout = nc.dram_tensor("wout", [128, QDIM], BF, kind="ExternalInput").ap()
    ident = nc.dram_tensor("ident", [128, 128], BF, kind="ExternalInput").ap()
    yT = nc.dram_tensor("yT", [QDIM, N], F32, kind="ExternalOutput").ap()

    xT_r = xT.rearrange("(k p) n -> p k n", p=128)
    wq_r = wq.rearrange("(k p) m -> p k m", p=128)
    wk_r = wk.rearrange("(k p) m -> p k m", p=128)
    wv_r = wv.rearrange("(k p) m -> p k m", p=128)
    wout_r = wout.rearrange("p (k f) -> p k f", f=128)
    yT_r = yT.rearrange("(m p) n -> m p n", p=128)

    # ---- static SBUF ----
    singles = ctx.enter_context(tc.tile_pool(name="singles", bufs=1))
    xT_sb = singles.tile([128, KT, N], BF, name="xT_sb", tag="xT_sb")
    wq_sb = singles.tile([128, KT, 128], BF, name="wq_sb", tag="wq_sb")
    wk_sb = singles.tile([128, KT, 128], BF, name="wk_sb", tag="wk_sb")
    wv_sb = singles.tile([128, KT, 128], BF, name="wv_sb", tag="wv_sb")
    wout_sb = singles.tile([128, KT, 128], BF, name="wout_sb", tag="wout_sb")
    id_sb = singles.tile([128, 128], BF, name="id_sb", tag="id_sb")
    qT_sb = singles.tile([128, N], BF, name="qT_sb", tag="qT_sb")
    kT_sb = singles.tile([128, N], BF, name="kT_sb", tag="kT_sb")
    # v per j-tile and head, with a trailing ones column: [j, jt, h, 0:64]=v,
    # [j, jt, h, 64]=1.0 (the AV rhs [j, 65] then accumulates Z in out col 64).
    # Double-buffered across KITER iterations so iteration n+1's v projection
    # does not serialize behind iteration n's AV reads.
    v_sb = singles.tile([128, 2, NJT, 2, 65], BF, name="v_sb", tag="v_sb")

    for k in range(KT):
        for q4 in range(4):
            qs = bass.ts(q4, N // 4)
            nc.sync.dma_start(out=xT_sb[:, k, qs], in_=xT_r[:, k, qs])
    nc.sync.dma_start(out=wq_sb, in_=wq_r)
    nc.sync.dma_start(out=wk_sb, in_=wk_r)
    nc.sync.dma_start(out=wv_sb, in_=wv_r)
    nc.sync.dma_start(out=wout_sb, in_=wout_r)
    nc.sync.dma_start(out=id_sb, in_=ident)
    nc.vector.memset(v_sb[:, :, :, :, 64], 1.0)

    psA = ctx.enter_context(tc.tile_pool(name="psA", bufs=3, space="PSUM"))
    psB = ctx.enter_context(tc.tile_pool(name="psB", bufs=2, space="PSUM"))
    ptp = ctx.enter_context(tc.tile_pool(name="ptp", bufs=6))
    sm = ctx.enter_context(tc.tile_pool(name="sm", bufs=2))
    yp = ctx.enter_context(tc.tile_pool(name="yp", bufs=3))

    from collections import deque
    fillers = deque()
    pending = []
    for _it in range(KITER):
        _compute(nc, psA, psB, ptp, sm, yp,
                 xT_sb, wq_sb, wk_sb, wv_sb, wout_sb, id_sb, qT_sb, kT_sb,
                 v_sb, yT_r, fillers, pending, _it)
    while fillers:
        fillers.popleft()[1]()
    while pending:
        if "tail" in ABLATE:
            break
        ich, oTs, tfn = pending.pop(0)
        for _, u in tfn(ich, oTs):
            u()


def _compute(nc, psA, psB, ptp, sm, yp, xT_sb, wq_sb, wk_sb, wv_sb,
             wout_sb, id_sb, qT_sb, kT_sb, v_sb, yT_r, fillers, pending, it):
    Exp = mybir.ActivationFunctionType.Exp
    Mul = mybir.AluOpType.mult
    Add = mybir.AluOpType.add
    I16 = mybir.dt.int16
    vb = it % 2

    # ---- 1-op Schraudolph exp on DVE for offloaded softmax tiles ----
    def dve_exp(st, out_ap):
        nc.vector.tensor_scalar(out=out_ap.bitcast(I16), in0=st,
                                scalar1=SCHR_A, scalar2=SCHR_B,
                                op0=Mul, op1=Add)

    # ---- projections: q^T, k^T = W^T @ x^T. Chunk 0's first S group needs
    # kT chunks 0-1 and qT chunk 0 (emitted inline); the other 13 chunks
    # drain as fillers, deadline-interleaved with the previous iteration's
    # leftover AV units (kT chunk 2q..2q+1 gates this chunk's quarter q;
    # AV(7) blocks of 8 gate the P^T quarter-slot reuse). ----
    def proj_qk(w_sb, dst, nch, nch2=None):
        # one "s" slot and one copy for up to two adjacent 512-col chunks
        pq = psA.tile([128, 1024], F32, tag="s", name="pq")
        chunks = (nch,) if nch2 is None else (nch, nch2)
        for ci, n in enumerate(chunks):
            for k in range(KT):
                nc.tensor.matmul(pq[:, ci * 512:ci * 512 + 512],
                                 lhsT=w_sb[:, k, :],
                                 rhs=xT_sb[:, k, bass.ts(n, 512)],
                                 start=(k == 0), stop=(k == KT - 1))
        cp = nc.scalar.copy if KCOPY == "act" else nc.vector.tensor_copy
        w = len(chunks) * 512
        cp(out=dst[:, nch * 512:nch * 512 + w], in_=pq[:, 0:w])

    if it == 0:
        # later iterations' kT0/kT1/qT0 recompute is hoisted into the
        # previous iteration's chunk-7 queue (prelude below)
        proj_qk(wk_sb, kT_sb, 0)
        proj_qk(wk_sb, kT_sb, 1)
        proj_qk(wq_sb, qT_sb, 0)
    punits = ([(2.6, lambda n=n: proj_qk(wk_sb, kT_sb, n, n + 1))
               for n in range(2, NCH, 2)]
              + [(2.6, lambda n=n: proj_qk(wq_sb, qT_sb, n, n + 1))
                 for n in range(1, NCH - 1, 2)]
              + [(1.3, lambda: proj_qk(wq_sb, qT_sb, NCH - 1))])
    left = list(fillers)
    fillers.clear()
    if len(left) >= 32:  # steady seam: [tail ...][AV(7) x32]
        tl, av = left[:len(left) - 32], left[len(left) - 32:]
        fillers.extend([punits[0]] + av[0:8]
                       + [punits[1], punits[2]] + av[8:16]
                       + [punits[3], punits[4]] + av[16:24]
                       + [punits[5]] + av[24:32] + tl + punits[6:])
    else:
        fillers.extend(left + punits)

    # ---- projection: v natural (x^T tiles stationary); emitted inside
    # i-chunk 0 per half, right before the AV that first consumes it ----
    def proj_v(jt4):
        # four j-tiles share one "s" slot and one copy
        pv = psA.tile([128, 1024], F32, tag="s", name="pv")
        for t in range(4):
            js = bass.ts(jt4 * 4 + t, 128)
            for k in range(KT):
                nc.tensor.matmul(pv[:, t * 128:t * 128 + 128],
                                 lhsT=xT_sb[:, k, js],
                                 rhs=wv_sb[:, k, :],
                                 start=(k == 0), stop=(k == KT - 1))
        cp = nc.scalar.copy if KCOPY == "act" else nc.vector.tensor_copy
        cp(out=v_sb[:, vb, jt4 * 4:jt4 * 4 + 4, :, 0:64],
           in_=pv[:, 0:512].rearrange("p (t h d) -> p t h d", t=4, h=2))

    # ---- tail: normalize by Z (PSUM col 64), transpose, out-project.
    # Returns a list of closures (filler units) so the PE work interleaves
    # with the next chunk's S/exp stream instead of blocking it. ----
    def tail_units(ich, oTs):
        ics = bass.ts(ich, 512)
        holder = {}

        def norm():
            oTn = holder["oTn"] = sm.tile([128, 4, 128], BF, tag="oTn",
                                          name="oTn")
            Mul = mybir.AluOpType.mult
            for hh in range(2):
                rz4 = sm.tile([128, 4], F32, tag="rz", name="rz4", bufs=4)
                nc.vector.reciprocal(out=rz4, in_=oTs[0][hh][:, :, 64])
                nc.vector.tensor_tensor(
                    out=oTn[:, :, hh * 64:hh * 64 + 64],
                    in0=oTs[0][hh][:, :, 0:64],
                    in1=rz4.rearrange("p (f o) -> p f o", o=1).broadcast_to([128, 4, 64]),
                    op=Mul)
            holder["osb"] = sm.tile([128, 512], BF, tag="osb", name="osb")

        def tr(isub):
            if KTRDMA:
                nc.sync.dma_start(out=holder["osb"][:, bass.ts(isub, 128)],
                                  in_=holder["oTn"][:, isub, :],
                                  transpose=True)
                return
            tps = psB.tile([128, 128], BF, tag="o", name="tps")
            nc.tensor.transpose(tps, holder["oTn"][:, isub, :], id_sb)
            cp = nc.scalar.copy if KCOPY == "act" else nc.vector.tensor_copy
            cp(out=holder["osb"][:, bass.ts(isub, 128)], in_=tps)

        def proj(m):
            py = psB.tile([128, 512], F32, tag="o", name="py")
            nc.tensor.matmul(py, lhsT=wout_sb[:, m, :], rhs=holder["osb"],
                             start=True, stop=True)
            yb = yp.tile([128, 512], F32, tag="yb", name="yb")
            if KYCOPY == "act":
                nc.scalar.copy(out=yb, in_=py)
            else:
                nc.vector.tensor_copy(out=yb, in_=py)
            nc.sync.dma_start(out=yT_r[m, :, ics], in_=yb)

        return ([(0.05, norm)]
                + [(0.2, lambda i=i: tr(i)) for i in range(4)]
                + [(0.3, lambda m=m: proj(m)) for m in range(KT)])

    # ---- AV: P^T stationary (lhsT), rhs = [v | 1], Z lands in out col 64.
    # One quarter-pass per unit: the 4 i-sub accumulation chains of a head
    # run sequentially (PSUM allows only one open accumulation group per
    # bank), then DVE folds the quarter's partial o^T into an SBUF
    # accumulator. The P^T quarter-tile is fully consumed after its two
    # units, releasing its slot early for the next chunk's exp. ----
    def av_units(qts, oSs):
        def avq(q, hh):
            if "av" in ABLATE:
                return
            if q == 0 and hh == 0:
                oSs.append([sm.tile([128, 4, 65], F32, tag="oS", name="oS",
                                    bufs=4)
                            for _ in range(2)])
                oSs.append([psB.tile([128, 4, 68], F32, tag="o", name="oT")
                            for _ in range(2)])
            oS, oT = oSs[0][hh], oSs[1][hh]
            for isub in range(4):
                io = hh * 512 + isub * 128
                for jl in range(8):
                    nc.tensor.matmul(
                        oT[:, isub, 0:65],
                        lhsT=qts[q][:, jl, io:io + 128],
                        rhs=v_sb[:, vb, q * 8 + jl, hh, :],
                        start=(jl == 0), stop=(jl == 7))
            if q == 0:
                cp = (nc.scalar.copy if KOSCOPY == "act"
                      else nc.vector.tensor_copy)
                cp(out=oS, in_=oT[:, :, 0:65])
            else:
                nc.vector.tensor_add(oS, oS, oT[:, :, 0:65])
        return [(1.0, lambda q=q, hh=hh: avq(q, hh))
                for q in range(4) for hh in range(2)]

    # ---- attention per i-chunk; AV(c) and tail(c-1) drain as fillers
    # between chunk c+1's S/exp groups, budgeted so the PE work emitted per
    # group stays within that group's ScalarE time (values in ~us of PE) ----
    def drain(budget):
        while fillers and budget > 0:
            cost, fn = fillers.popleft()
            fn()
            budget -= cost

    for ich in range(NCH):
        ics = bass.ts(ich, 512)
        drain(KSEAM)
        qts = []
        for q in range(4):
            ptb = ptp.tile([128, 8, 1024], BF, tag="pt", name="ptb")
            ptf = ptb.rearrange("p a b -> p (a b)")
            qts.append(ptb)
            g = 0
            for gi, csz in enumerate((2,) * 8):
                st = psA.tile([128, csz * 512], F32, tag="s", name="st")
                for s in range(csz):
                    jl, h = (g + s) // 2, (g + s) % 2
                    jt = q * 8 + jl
                    js = bass.ts(jt, 128)
                    if "s" in ABLATE:
                        continue
                    nc.tensor.matmul(st[:, bass.ts(s, 512)],
                                     lhsT=kT_sb[h * 64:h * 64 + 64, js],
                                     rhs=qT_sb[h * 64:h * 64 + 64, ics],
                                     start=True, stop=True,
                                     tile_position=(h * 64, 0))
                if "exp" not in ABLATE:
                    dst = ptf[:, g * 512:(g + csz) * 512]
                    if KSPLIT and "dve" not in ABLATE:
                        nc.scalar.activation(out=dst[:, 0:KSPLIT],
                                             in_=st[:, 0:KSPLIT],
                                             func=Exp, scale=SCALE)
                        dve_exp(st[:, KSPLIT:csz * 512],
                                dst[:, KSPLIT:csz * 512])
                    elif gi in DVE_GI and "dve" not in ABLATE:
                        dve_exp(st, dst)
                    else:
                        nc.scalar.activation(out=dst, in_=st,
                                             func=Exp, scale=SCALE)
                g += csz
                drain(0.55)
            if ich == NCH - 1 and q == 0:
                # prelude: next iteration's first projections, emitted after
                # this iteration's last reader of kT chunks 0-1 / qT chunk 0
                fillers.extendleft([
                    (1.3, lambda: proj_qk(wq_sb, qT_sb, 0)),
                    (2.6, lambda: proj_qk(wk_sb, kT_sb, 0, 1))])
        if pending and "tail" not in ABLATE:
            pich, poTs, ptfn = pending.pop(0)
            fillers.extend(ptfn(pich, poTs))
        oSs = []
        avs = av_units(qts, oSs)
        if ich == 0:
            # refresh v (quarter q's 8 tiles) right before the first AV
            # quarter-pass that consumes it
            avs = [u for q in range(4)
                   for u in ([(1.7, lambda jt4=jt4: proj_v(jt4))
                              for jt4 in (q * 2, q * 2 + 1)]
                             + avs[2 * q:2 * q + 2])]
        fillers.extend(avs)
        pending.append((ich, oSs, tail_units))


_CACHE = {}


def _build():
    if "nc" not in _CACHE:
        nc = bacc.Bacc("TRN2", target_bir_lowering=False, debug=False,
                       num_devices=8)
        from contextlib import ExitStack
        with tile.TileContext(nc) as tc:
            with ExitStack() as ctx:
                _body(ctx, tc)
        nc.compile()
        _CACHE["nc"] = nc
    return _CACHE["nc"]


def make_in_maps(x, Wq, Wk, Wv, Wout):
    in_maps = []
    ident = np.eye(128, dtype=BF16)
    for core in range(8):
        b = core // 4
        sl = slice((core % 4) * 128, (core % 4) * 128 + 128)
        in_maps.append({
            "xT": x[b].T.astype(BF16),
            "wq": Wq[:, sl].astype(BF16),
            "wk": Wk[:, sl].astype(BF16),
            "wv": Wv[:, sl].astype(BF16),
            "wout": Wout[sl, :].astype(BF16),
            "ident": ident,
        })
    return in_maps


def kernel(x, Wq, Wk, Wv, Wout, b_out):
    x, Wq, Wk, Wv, Wout, b_out = (np.asarray(a) for a in
                                  (x, Wq, Wk, Wv, Wout, b_out))
    nc = _build()
    in_maps = make_in_maps(x, Wq, Wk, Wv, Wout)
    res = run_bass_kernel_spmd(nc, in_maps, core_ids=list(range(8)))
    y = np.zeros((B, N, QDIM), np.float32)
    for core in range(8):
        y[core // 4] += res.results[core]["yT"].T
    y += b_out.astype(np.float32)
    return y

